# revision 8
# baseline (speedup 1.0000x reference)
"""Pairwise cosine-similarity scorer (CosScorer) for Trainium2.

Full-input contract: kernel(xs_pad=[8,8192,256] f32, spk_emb=[8,200,256] f32)
-> [8,8192,200] f32, computed as dot(x,y)/max(||x||*||y||, eps).

Sharding: data-parallel over B — core i handles batch element i (B=8 on
8 cores), SPMD program, no collectives.

Per-core pipeline (x=[8192,256], spk=[200,256] -> out=[8192,200]):
  - All matmuls in bf16 (tolerance 2e-2 vs ~2e-3 bf16 dot error), PSUM
    accumulates fp32. fp32 matmul streams at 4 cyc/row + 2-pass
    LDWEIGHTS; bf16 is 1 cyc/row — PE drops from ~71us floor to ~17us.
  - x transposes moved OFF the PE onto the DMA transpose XBAR
    (InstDmaTransposeAnt, 16-bit, 14ns per 16x128 tile): the ACT engine
    converts each x macro [128,4x256] f32 -> bf16 writing an interleaved
    layout (col j = dd*8+n*2+c), so ONE transpose DMA per macro yields
    xts[dd, n*2+c, t] ready to be matmul lhsT. spk is transposed the
    same way (one-time).
  - Norms stay fp32: two multi-group bn_stats per macro (free<=512 HW
    limit) + per-group stats math on DVE; sqrt on ACT; reciprocal DVE.
    1/||spk|| is folded into spknT, 1/||x|| into the PSUM->SBUF output
    copies (3 on ACT, 1 on DVE per macro to balance engines).
  - Software-pipelined emission: output copies + store for macro i-1 are
    emitted while macro i computes, so strict per-engine FIFOs never
    head-of-line block on the PE. PSUM pool = 8 banks = 2 macros.
  - Rings: x loads on SyncE HWDGE; transpose DMAs + output stores on
    ScalarE HWDGE.

Roofline: 15.1MB HBM traffic/core at ~360GB/s => ~42us floor; every
engine budget sits at 2.2-2.7us per 2.59us macro pace.
"""

import sys

if "/opt/trn_rl_repo" not in sys.path:
    sys.path.insert(0, "/opt/trn_rl_repo")

import numpy as np

B, T, S, D = 8, 8192, 200, 256
P = 128
NSUB = 4            # 128-row subtiles per macro
NMACRO = T // (P * NSUB)
NCHUNK = D // P     # contraction chunks

_CACHE = {}


def _build():
    if "nc" in _CACHE:
        return _CACHE["nc"]

    from contextlib import ExitStack

    import concourse.tile as tile
    from concourse import bacc, mybir

    f32 = mybir.dt.float32
    bf16 = mybir.dt.bfloat16
    Act = mybir.ActivationFunctionType

    nc = bacc.Bacc("TRN2", target_bir_lowering=False, debug=False)
    x = nc.dram_tensor("x", [T, D], f32, kind="ExternalInput").ap()
    spk = nc.dram_tensor("spk", [S, D], f32, kind="ExternalInput").ap()
    out = nc.dram_tensor("out", [T, S], f32, kind="ExternalOutput").ap()

    with tile.TileContext(nc) as tc, ExitStack() as ctx:
        const = ctx.enter_context(tc.tile_pool(name="const", bufs=1))
        xin = ctx.enter_context(tc.tile_pool(name="xin", bufs=3))
        xbp = ctx.enter_context(tc.tile_pool(name="xbp", bufs=3))
        xtp = ctx.enter_context(tc.tile_pool(name="xtp", bufs=3))
        stats = ctx.enter_context(tc.tile_pool(name="stats", bufs=3))
        outp = ctx.enter_context(tc.tile_pool(name="outp", bufs=3))
        psum_o = ctx.enter_context(tc.tile_pool(name="psum_o", bufs=8, space="PSUM"))

        # t = i*512 + n*128 + p
        x_r = x.rearrange("(i n p) d -> i p n d", p=P, n=NSUB)
        out_r = out.rearrange("(i n p) s -> i p n s", p=P, n=NSUB)

        # ---- spk loads first (sync ring), then x prefetch ----
        sp_tiles = []
        for s0, ps in ((0, P), (P, S - P)):
            sp = const.tile([P, D], f32, tag=f"sp{s0}", name=f"sp{s0}")
            nc.sync.dma_start(out=sp[:ps], in_=spk[s0 : s0 + ps])
            sp_tiles.append(sp)

        def emit_load(i):
            xm = xin.tile([P, NSUB, D], f32, tag="xm", name=f"xm{i}")
            nc.sync.dma_start(out=xm, in_=x_r[i])
            return xm

        xm_t = {0: emit_load(0), 1: emit_load(1)}

        # pre-warm the Sqrt ACT table while DMAs run (table load ~2.7us)
        warm = const.tile([P, 1], f32, tag="warm")
        nc.vector.memset(warm, 1.0)
        nc.scalar.sqrt(warm, warm)

        # ---- spk prep: normalized bf16, DMA-transposed to [dd, c, s] ----
        # spknT[dd, c, s] = spk[s, c*128+dd] / ||spk[s]||
        spknT = const.tile([P, NCHUNK, 2 * P], bf16, tag="spknT", name="spknT")
        for (s0, ps), sp in zip(((0, P), (P, S - P)), sp_tiles):
            ssq = const.tile([P, 1], f32, tag=f"ssq{s0}")
            sq = const.tile([P, D], f32, tag=f"sq{s0}")
            nc.scalar.activation(
                out=sq[:ps], in_=sp[:ps], func=Act.Square, accum_out=ssq[:ps]
            )
            nc.scalar.sqrt(ssq[:ps], ssq[:ps])
            nc.vector.reciprocal(ssq[:ps], ssq[:ps])
            # normalized bf16 spk rows (plain layout)
            spn = const.tile([P, D], bf16, tag=f"spn{s0}")
            if ps < P:
                nc.vector.memset(spn, 0.0)
            nc.vector.tensor_scalar_mul(
                out=spn[:ps], in0=sp[:ps], scalar1=ssq[:ps]
            )
            # XBAR transpose is chunk-major: out[dd, c, s] = spn[s, c*128+dd]
            nc.sync.dma_start(
                out=spknT[:, :, s0 : s0 + P], in_=spn, transpose=True
            )

        def emit_convert(i, xm):
            # f32 -> bf16 via gpsimd casting DMA (software DGE): zero
            # compute-engine cycles, DMA engines do the conversion
            xb = xbp.tile([P, NSUB, D], bf16, tag="xb", name=f"xb{i}")
            nc.gpsimd.dma_start(out=xb, in_=xm)
            return xb

        def emit_transpose(i, xb):
            # one DMA-transpose per macro, chunk-major XBAR semantics:
            # xts[dd, n*2+c, t] = xb[t, (n*2+c)*128+dd] = x[t, n, c*128+dd]
            xts = xtp.tile([P, NSUB * NCHUNK, P], bf16, tag="xts", name=f"xts{i}")
            nc.sync.dma_start(
                out=xts, in_=xb.rearrange("p n d -> p (n d)"), transpose=True
            )
            return xts

        def emit_norms(i, xm):
            stt = stats.tile([P, NSUB, 6], f32, tag="stt", name=f"stt{i}")
            mv = stats.tile([P, NSUB, 2], f32, tag="mv", name=f"mv{i}")
            msq = stats.tile([P, NSUB], f32, tag="msq")
            ssq = stats.tile([P, NSUB], f32, tag="ssq")
            inv = stats.tile([P, NSUB], f32, tag="inv", name=f"inv{i}")
            # backend BIR verifier requires one bn group per instruction
            for n in range(NSUB):
                nc.vector.bn_stats(out=stt[:, n, :], in_=xm[:, n])
                nc.vector.bn_aggr(out=mv[:, n, :], in_=stt[:, n, :])
            # sumsq = (var + mean^2); norm = sqrt(D * sumsq)
            nc.vector.tensor_mul(msq, mv[:, :, 0], mv[:, :, 0])
            nc.vector.tensor_add(ssq, msq, mv[:, :, 1])
            nc.scalar.activation(out=ssq, in_=ssq, func=Act.Sqrt, scale=float(D))
            nc.vector.reciprocal(inv, ssq)
            return inv

        def emit_scores(i, xts):
            psos = []
            for n in range(NSUB):
                pso = psum_o.tile([P, S], f32, tag="pso", name=f"pso{i}_{n}")
                for c in range(NCHUNK):
                    nc.tensor.matmul(
                        pso,
                        lhsT=xts[:, n * NCHUNK + c, :],
                        rhs=spknT[:, c, 0:S],
                        start=(c == 0),
                        stop=(c == NCHUNK - 1),
                    )
                psos.append(pso)
            return psos

        def emit_out(i, psos, inv):
            omac = outp.tile([P, NSUB, S], f32, tag="omac", name=f"omac{i}")
            # fused *1/||x|| PSUM->SBUF copies, all on ACT (DVE is norm-bound)
            for n in range(NSUB):
                nc.scalar.mul(omac[:, n, :], psos[n], inv[:, n : n + 1])
            nc.sync.dma_start(out=out_r[i], in_=omac)

        # ---- software-pipelined main loop (copies lag one macro) ----
        prev = None
        for i in range(NMACRO):
            if i + 2 < NMACRO:
                xm_t[i + 2] = emit_load(i + 2)
            xm = xm_t.pop(i)
            xb = emit_convert(i, xm)
            xts = emit_transpose(i, xb)
            if prev is not None:
                emit_out(i - 1, prev[0], prev[1])
            inv = emit_norms(i, xm)
            psos = emit_scores(i, xts)
            prev = (psos, inv)
        emit_out(NMACRO - 1, prev[0], prev[1])

    nc.compile()
    _CACHE["nc"] = nc
    return nc


def _run(xs_pad, spk_emb, trace=False):
    from concourse.bass_utils import run_bass_kernel_spmd

    nc = _build()
    xs_pad = np.ascontiguousarray(np.asarray(xs_pad), dtype=np.float32)
    spk_emb = np.ascontiguousarray(np.asarray(spk_emb), dtype=np.float32)
    assert xs_pad.shape == (B, T, D) and spk_emb.shape == (B, S, D)
    in_maps = [{"x": xs_pad[i], "spk": spk_emb[i]} for i in range(B)]
    res = run_bass_kernel_spmd(nc, in_maps, list(range(B)), trace=trace)
    out = np.stack([res.results[i]["out"] for i in range(B)], axis=0)
    return out, res


def kernel(xs_pad, spk_emb):
    out, _ = _run(xs_pad, spk_emb, trace=False)
    return out


# revision 11
# speedup vs baseline: 1.0228x; 1.0228x over previous
"""Pairwise cosine-similarity scorer (CosScorer) for Trainium2.

Full-input contract: kernel(xs_pad=[8,8192,256] f32, spk_emb=[8,200,256] f32)
-> [8,8192,200] f32, computed as dot(x,y)/max(||x||*||y||, eps).

Sharding: data-parallel over B — core i handles batch element i (B=8 on
8 cores), SPMD program, no collectives.

Per-core pipeline (x=[8192,256], spk=[200,256] -> out=[8192,200]):
  - All matmuls in bf16 (tolerance 2e-2 vs ~2e-3 bf16 dot error), PSUM
    accumulates fp32. fp32 matmul streams at 4 cyc/row + 2-pass
    LDWEIGHTS; bf16 is 1 cyc/row — PE drops from ~71us floor to ~17us.
  - x transposes moved OFF the PE onto the DMA transpose XBAR
    (InstDmaTransposeAnt, 16-bit, 14ns per 16x128 tile): the ACT engine
    converts each x macro [128,4x256] f32 -> bf16 writing an interleaved
    layout (col j = dd*8+n*2+c), so ONE transpose DMA per macro yields
    xts[dd, n*2+c, t] ready to be matmul lhsT. spk is transposed the
    same way (one-time).
  - Norms stay fp32: two multi-group bn_stats per macro (free<=512 HW
    limit) + per-group stats math on DVE; sqrt on ACT; reciprocal DVE.
    1/||spk|| is folded into spknT, 1/||x|| into the PSUM->SBUF output
    copies (3 on ACT, 1 on DVE per macro to balance engines).
  - Software-pipelined emission: output copies + store for macro i-1 are
    emitted while macro i computes, so strict per-engine FIFOs never
    head-of-line block on the PE. PSUM pool = 8 banks = 2 macros.
  - Rings: x loads on SyncE HWDGE; transpose DMAs + output stores on
    ScalarE HWDGE.

Roofline: 15.1MB HBM traffic/core at ~360GB/s => ~42us floor; every
engine budget sits at 2.2-2.7us per 2.59us macro pace.
"""

import sys

if "/opt/trn_rl_repo" not in sys.path:
    sys.path.insert(0, "/opt/trn_rl_repo")

import numpy as np

B, T, S, D = 8, 8192, 200, 256
P = 128
NSUB = 4            # 128-row subtiles per macro
NMACRO = T // (P * NSUB)
NCHUNK = D // P     # contraction chunks

_CACHE = {}


def _build():
    if "nc" in _CACHE:
        return _CACHE["nc"]

    from contextlib import ExitStack

    import concourse.tile as tile
    from concourse import bacc, mybir

    f32 = mybir.dt.float32
    bf16 = mybir.dt.bfloat16
    Act = mybir.ActivationFunctionType

    nc = bacc.Bacc("TRN2", target_bir_lowering=False, debug=False)
    x = nc.dram_tensor("x", [T, D], f32, kind="ExternalInput").ap()
    spk = nc.dram_tensor("spk", [S, D], f32, kind="ExternalInput").ap()
    out = nc.dram_tensor("out", [T, S], f32, kind="ExternalOutput").ap()

    with tile.TileContext(nc) as tc, ExitStack() as ctx:
        const = ctx.enter_context(tc.tile_pool(name="const", bufs=1))
        xin = ctx.enter_context(tc.tile_pool(name="xin", bufs=3))
        xbp = ctx.enter_context(tc.tile_pool(name="xbp", bufs=3))
        xtp = ctx.enter_context(tc.tile_pool(name="xtp", bufs=3))
        stats = ctx.enter_context(tc.tile_pool(name="stats", bufs=3))
        outp = ctx.enter_context(tc.tile_pool(name="outp", bufs=3))
        psum_o = ctx.enter_context(tc.tile_pool(name="psum_o", bufs=8, space="PSUM"))

        # t = i*512 + n*128 + p
        x_r = x.rearrange("(i n p) d -> i p n d", p=P, n=NSUB)
        out_r = out.rearrange("(i n p) s -> i p n s", p=P, n=NSUB)

        # ---- spk loads first (sync ring), then x prefetch ----
        sp_tiles = []
        for s0, ps in ((0, P), (P, S - P)):
            sp = const.tile([P, D], f32, tag=f"sp{s0}", name=f"sp{s0}")
            nc.sync.dma_start(out=sp[:ps], in_=spk[s0 : s0 + ps])
            sp_tiles.append(sp)

        def emit_load(i):
            xm = xin.tile([P, NSUB, D], f32, tag="xm", name=f"xm{i}")
            nc.sync.dma_start(out=xm, in_=x_r[i])
            return xm

        xm_t = {0: emit_load(0), 1: emit_load(1)}

        # pre-warm the Sqrt ACT table while DMAs run (table load ~2.7us)
        warm = const.tile([P, 1], f32, tag="warm")
        nc.vector.memset(warm, 1.0)
        nc.scalar.sqrt(warm, warm)

        # ---- spk prep: normalized bf16, DMA-transposed to [dd, c, s] ----
        # spknT[dd, c, s] = spk[s, c*128+dd] / ||spk[s]||
        spknT = const.tile([P, NCHUNK, 2 * P], bf16, tag="spknT", name="spknT")
        for (s0, ps), sp in zip(((0, P), (P, S - P)), sp_tiles):
            ssq = const.tile([P, 1], f32, tag=f"ssq{s0}")
            sq = const.tile([P, D], f32, tag=f"sq{s0}")
            nc.scalar.activation(
                out=sq[:ps], in_=sp[:ps], func=Act.Square, accum_out=ssq[:ps]
            )
            nc.scalar.sqrt(ssq[:ps], ssq[:ps])
            nc.vector.reciprocal(ssq[:ps], ssq[:ps])
            # normalized bf16 spk rows (plain layout)
            spn = const.tile([P, D], bf16, tag=f"spn{s0}")
            if ps < P:
                nc.vector.memset(spn, 0.0)
            nc.vector.tensor_scalar_mul(
                out=spn[:ps], in0=sp[:ps], scalar1=ssq[:ps]
            )
            # XBAR transpose is chunk-major: out[dd, c, s] = spn[s, c*128+dd]
            nc.sync.dma_start(
                out=spknT[:, :, s0 : s0 + P], in_=spn, transpose=True
            )

        def emit_convert(i, xm):
            # f32 -> bf16 via gpsimd casting DMA (software DGE): zero
            # compute-engine cycles, DMA engines do the conversion
            xb = xbp.tile([P, NSUB, D], bf16, tag="xb", name=f"xb{i}")
            nc.gpsimd.dma_start(out=xb, in_=xm)
            return xb

        def emit_transpose(i, xb):
            # one DMA-transpose per macro, chunk-major XBAR semantics:
            # xts[dd, n*2+c, t] = xb[t, (n*2+c)*128+dd] = x[t, n, c*128+dd]
            xts = xtp.tile([P, NSUB * NCHUNK, P], bf16, tag="xts", name=f"xts{i}")
            nc.sync.dma_start(
                out=xts, in_=xb.rearrange("p n d -> p (n d)"), transpose=True
            )
            return xts

        def emit_norms(i, xm):
            stt = stats.tile([P, NSUB, 6], f32, tag="stt", name=f"stt{i}")
            mv = stats.tile([P, NSUB, 2], f32, tag="mv", name=f"mv{i}")
            msq = stats.tile([P, NSUB], f32, tag="msq")
            ssq = stats.tile([P, NSUB], f32, tag="ssq")
            inv = stats.tile([P, NSUB], f32, tag="inv", name=f"inv{i}")
            # backend BIR verifier requires one bn group per instruction
            for n in range(NSUB):
                nc.vector.bn_stats(out=stt[:, n, :], in_=xm[:, n])
                nc.vector.bn_aggr(out=mv[:, n, :], in_=stt[:, n, :])
            # sumsq = (var + mean^2); inv = 1/sqrt(D*sumsq) = sqrt((1/sumsq)/D)
            # recip runs BEFORE sqrt so the whole DVE chain is engine-local
            # (no DVE->ACT->DVE ping-pong serializing consecutive macros);
            # ACT's sqrt directly produces inv for its own copies.
            nc.vector.tensor_mul(msq, mv[:, :, 0], mv[:, :, 0])
            nc.vector.tensor_add(ssq, msq, mv[:, :, 1])
            nc.vector.reciprocal(ssq, ssq)
            nc.scalar.activation(
                out=inv, in_=ssq, func=Act.Sqrt, scale=1.0 / float(D)
            )
            return inv

        def emit_scores(i, xts):
            psos = []
            for n in range(NSUB):
                pso = psum_o.tile([P, S], f32, tag="pso", name=f"pso{i}_{n}")
                for c in range(NCHUNK):
                    nc.tensor.matmul(
                        pso,
                        lhsT=xts[:, n * NCHUNK + c, :],
                        rhs=spknT[:, c, 0:S],
                        start=(c == 0),
                        stop=(c == NCHUNK - 1),
                    )
                psos.append(pso)
            return psos

        def emit_out(i, psos, inv):
            omac = outp.tile([P, NSUB, S], f32, tag="omac", name=f"omac{i}")
            # fused *1/||x|| PSUM->SBUF copies, all on ACT (DVE is norm-bound)
            for n in range(NSUB):
                nc.scalar.mul(omac[:, n, :], psos[n], inv[:, n : n + 1])
            nc.sync.dma_start(out=out_r[i], in_=omac)

        # ---- software-pipelined main loop (copies lag one macro) ----
        prev = None
        for i in range(NMACRO):
            if i + 2 < NMACRO:
                xm_t[i + 2] = emit_load(i + 2)
            xm = xm_t.pop(i)
            xb = emit_convert(i, xm)
            xts = emit_transpose(i, xb)
            if prev is not None:
                emit_out(i - 1, prev[0], prev[1])
            inv = emit_norms(i, xm)
            psos = emit_scores(i, xts)
            prev = (psos, inv)
        emit_out(NMACRO - 1, prev[0], prev[1])

    nc.compile()
    _CACHE["nc"] = nc
    return nc


def _run(xs_pad, spk_emb, trace=False):
    from concourse.bass_utils import run_bass_kernel_spmd

    nc = _build()
    xs_pad = np.ascontiguousarray(np.asarray(xs_pad), dtype=np.float32)
    spk_emb = np.ascontiguousarray(np.asarray(spk_emb), dtype=np.float32)
    assert xs_pad.shape == (B, T, D) and spk_emb.shape == (B, S, D)
    in_maps = [{"x": xs_pad[i], "spk": spk_emb[i]} for i in range(B)]
    res = run_bass_kernel_spmd(nc, in_maps, list(range(B)), trace=trace)
    out = np.stack([res.results[i]["out"] for i in range(B)], axis=0)
    return out, res


def kernel(xs_pad, spk_emb):
    out, _ = _run(xs_pad, spk_emb, trace=False)
    return out
